# revision 1
# baseline (speedup 1.0000x reference)
"""Trainium2 Bass kernel for CSHA attention (ECA channel attention + spatial attention).

Computes, for x [B, C, H, W] = [32, 256, 64, 64]:
    out = x * (1 + ch_w[c] + sp[h, w])
where
    ch_w = sigmoid(conv1d_k5(mean_hw(x)))          (per-channel, ECA)
    sp   = sigmoid(conv2d_7x7([max_c(x); mean_c(x)]) + b)   (per-pixel)

Strategy: pure data parallel over batch across 8 NeuronCores (4 batches per
core).  Per core the 16.8MB shard is kept resident in SBUF; stats are
computed with a mix of PE matmuls (channel sums, broadcasts, transposes,
conv-as-banded-matmul), ACT (free-dim sums via accum_out, sigmoids) and DVE
(channel max via fold+transpose+reduce, final fused multiply).
"""

import os
import sys

import numpy as np

sys.path.insert(0, "/opt/trn_rl_repo")

B, C, H, W = 32, 256, 64, 64
HW = H * W            # 4096
N_CORES = 8
BPC = B // N_CORES    # 4 batches per core
H2 = H // 2           # 32 "h2" column blocks (hw = h2*128 + (h%2)*64 + w)


# ---------------------------------------------------------------------------
# Host-side constant building (tiny, from the conv weights)
# ---------------------------------------------------------------------------

def _build_host_consts(conv1d_w, conv2d_w, conv2d_b):
    """Build matmul-form weight matrices from the conv weights.

    Returns
      w1t  [128, 2, 256] f32 : ECA conv1d as banded matrix, lhsT layout.
                               w1t[cl, kh, co] = W1[co, kh*128+cl] where
                               W1 @ (channel sums) = conv1d(mean) (1/HW folded).
      wd   [128, 10, 128] f32: spatial conv2d as 10 accumulated matmuls in the
                               interleaved layout (partition = (h%2)*64 + w,
                               free = h//2).  wd[:, ch*5+di, :] is the lhsT for
                               channel ch (0=max pool, 1=avg pool; 1/C folded
                               into ch=1) and h2-shift delta = di-2.
      bias [128, 1] f32      : conv2d bias replicated.
    """
    w5 = np.asarray(conv1d_w, np.float32)[0, 0]           # [5]
    W1 = np.zeros((C, C), np.float32)
    for k in range(5):
        co = np.arange(C)
        ci = co + k - 2
        m = (ci >= 0) & (ci < C)
        W1[co[m], ci[m]] = w5[k] / HW
    w1t = W1.T.reshape(2, 128, C).transpose(1, 0, 2).astype(np.float16)  # [128, 2, 256]

    wt = np.asarray(conv2d_w, np.float32)[0].copy()       # [2, 7, 7] (ch, dy, dx)
    wt[1] /= C
    wd = np.zeros((128, 10, 128), np.float32)
    wi = np.arange(64)
    for ch in range(2):
        for di, d in enumerate(range(-2, 3)):
            M = np.zeros((128, 128), np.float32)
            for hp in range(2):
                for hpp in range(2):
                    dy = 2 * d + hp - hpp + 3
                    if not (0 <= dy <= 6):
                        continue
                    # band over w: M[hp*64+w_in, hpp*64+w_out] = wt[ch,dy,w_in-w_out+3]
                    for dx in range(7):
                        w_out = wi
                        w_in = w_out + dx - 3
                        msk = (w_in >= 0) & (w_in < 64)
                        M[hp * 64 + w_in[msk], hpp * 64 + w_out[msk]] = wt[ch, dy, dx]
            wd[:, ch * 5 + di, :] = M
    bias = np.full((128, 1), float(np.asarray(conv2d_b)[0]), np.float32)
    return w1t, wd.astype(np.float16), bias


# ---------------------------------------------------------------------------
# Device kernel (per core; SPMD over 8 cores)
# ---------------------------------------------------------------------------

def _build_nc():
    import ml_dtypes
    import concourse.bass as bass
    import concourse.tile as tile
    from concourse import mybir

    f32 = mybir.dt.float32
    f32r = mybir.dt.float32r
    bf16 = mybir.dt.bfloat16
    f16 = mybir.dt.float16

    nc = bass.Bass()

    xs_d = nc.dram_tensor("xs", [BPC, C, H, W], f32, kind="ExternalInput")
    w1t_d = nc.dram_tensor("w1t", [128, 2, C], f16, kind="ExternalInput")
    wd_d = nc.dram_tensor("wd", [128, 10, 128], f16, kind="ExternalInput")
    bias_d = nc.dram_tensor("bias", [128, 1], f32, kind="ExternalInput")
    out_d = nc.dram_tensor("out", [BPC, C, H, W], f32, kind="ExternalOutput")

    i128b_d = nc.inline_tensor(np.eye(128, dtype=np.float16), "i128b")
    ones32_d = nc.inline_tensor(np.ones((128, 32), np.float16), "ones32")
    ones1_d = nc.inline_tensor(np.ones((2, 128), np.float16), "ones1")

    AX = mybir.AxisListType
    ALU = mybir.AluOpType
    ACT = mybir.ActivationFunctionType

    with tile.TileContext(nc) as tc:
        with (
            tc.tile_pool(name="consts", bufs=1) as consts,
            tc.tile_pool(name="xp", bufs=1) as xp,
            tc.tile_pool(name="work", bufs=2) as work,
            tc.tile_pool(name="m1p", bufs=1) as m1p,
            tc.tile_pool(name="scrp", bufs=1) as scrp,
            tc.tile_pool(name="flatp", bufs=1) as flatp,
            tc.tile_pool(name="psb", bufs=3, space=bass.MemorySpace.PSUM) as psb,
            tc.tile_pool(name="pss", bufs=2, space=bass.MemorySpace.PSUM) as pss,
        ):
            # ---- constants to SBUF ----
            w1t_t = consts.tile([128, 2, C], f16)
            nc.sync.dma_start(out=w1t_t, in_=w1t_d[:])
            wd_t = consts.tile([128, 10, 128], f16)
            nc.sync.dma_start(out=wd_t, in_=wd_d[:])
            bias_t = consts.tile([128, 1], f32)
            nc.sync.dma_start(out=bias_t, in_=bias_d[:])
            i128b_t = consts.tile([128, 128], f16)
            nc.sync.dma_start(out=i128b_t, in_=i128b_d[:])
            ones32_t = consts.tile([128, 32], f16)
            nc.sync.dma_start(out=ones32_t, in_=ones32_d[:])
            ones1_t = consts.tile([2, 128], f16)
            nc.sync.dma_start(out=ones1_t, in_=ones1_d[:])

            # Dummy matmuls: absorb the const-load DMA waits on PE early so
            # steady-state (ldweights, matmul) pairs stay within the 2-wait
            # hardware budget.
            pd = pss.tile([1, 8], f32, tag="sm")
            for k, (lhs, rhs) in enumerate((
                (w1t_t[:, 0, 0:1], w1t_t[:, 0, 1:2]),
                (wd_t[:, 0, 0:1], wd_t[:, 0, 1:2]),
                (i128b_t[:, 0:1], i128b_t[:, 1:2]),
                (ones32_t[:, 0:1], ones32_t[:, 1:2]),
                (ones1_t[:, 0:1], ones1_t[:, 1:2]),
            )):
                nc.tensor.matmul(pd[:, k : k + 1], lhs, rhs, start=True, stop=True)

            x_tiles = []
            for b in range(BPC):
                x_t = xp.tile([128, 2, HW], f32, tag=f"x{b}")
                x_tiles.append(x_t)
                # x[b, g*128+cl, hw] -> x_t[cl, g, hw]
                nc.sync.dma_start(
                    out=x_t, in_=xs_d[b].rearrange("(g c) h w -> c g (h w)", g=2)
                )

            # absorb the bias-load DMA wait on ACT
            junk0 = work.tile([128, 1], f32, tag="junk")
            nc.scalar.activation(out=junk0, in_=bias_t, func=ACT.Copy)

            for b in range(BPC):
                x_t = x_tiles[b]

                # absorb this x tile's DMA wait on ACT before the ECA pass
                junk = work.tile([1, 16], f32, tag="junk")
                nc.scalar.activation(out=junk, in_=x_t[0:1, 0, 0:16], func=ACT.Copy)
                junkv = work.tile([1, 16], f32, tag="junkv")
                nc.vector.tensor_copy(junkv, x_t[0:1, 1, 0:16])

                # ---- ECA channel sums (ACT f16 cast copy + fp32 accum_out) ----
                scr = scrp.tile([128, 2, HW], f16, tag="scr")
                yb = work.tile([128, 2], f32, tag="y", bufs=3)
                for h in range(2):
                    nc.scalar.activation(
                        out=scr[:, h, :],
                        in_=x_t[:, h, :],
                        func=ACT.Copy,
                        accum_out=yb[:, h : h + 1],
                    )

                # ---- channel sums over c (PE, f16) -> interleaved ap map ----
                cs_full = work.tile([128, 2, 512], f16, tag="cs_sb", bufs=3)
                for i in range(2):
                    psc = pss.tile([128, 512], f32, tag="sm")
                    for q in range(4):
                        j = i * 4 + q
                        o = psc[32 * q : 32 * q + 32, :]
                        for h in range(2):
                            nc.tensor.matmul(
                                o,
                                ones32_t,
                                scr[:, h, j * 512 : (j + 1) * 512],
                                start=(h == 0),
                                stop=(h == 1),
                                tile_position=(0, 32 * q),
                            )
                    nc.scalar.activation(
                        out=cs_full[:, i, :], in_=psc, func=ACT.Copy
                    )

                ap_map = work.tile([128, 36], f16, tag="ap", bufs=3)
                nc.vector.memset(ap_map.rearrange("p (a b) -> p a b", a=18)[:, 0:18:17, :], 0.0)
                pfull = pss.tile([128, 2, 4, 128], f16, tag="sm")
                for i in range(2):
                    for s1 in range(4):
                        nc.tensor.transpose(
                            out=pfull[:, i, s1, :],
                            in_=cs_full[:, i, s1 * 128 : (s1 + 1) * 128],
                            identity=i128b_t,
                        )
                # ap_map col h2 = (i*4+q)*4 + s1  <-  pfull[:, i, s1, 32*q]
                nc.vector.tensor_copy(
                    out=ap_map[:, 2:34].rearrange("p (i q s) -> p i s q", i=2, q=4),
                    in_=pfull[:, :, :, 0:97:32],
                )

                # ---- channel max: fold halves (DVE), transpose (PE), reduce ----
                m1 = m1p.tile([128, HW], f16, tag="m1")
                nc.vector.tensor_max(m1, scr[:, 0, :], scr[:, 1, :])
                mp_map = work.tile([128, 36], f16, tag="mp", bufs=3)
                nc.vector.memset(mp_map.rearrange("p (a b) -> p a b", a=18)[:, 0:18:17, :], 0.0)
                for t in range(4):
                    pmt = psb.tile([128, 8, 128], f16, tag="big")
                    for k in range(8):
                        nc.tensor.transpose(
                            out=pmt[:, k, :],
                            in_=m1[:, (t * 8 + k) * 128 : (t * 8 + k + 1) * 128],
                            identity=i128b_t,
                        )
                    nc.vector.reduce_max(
                        out=mp_map[:, 2 + t * 8 : 2 + t * 8 + 8], in_=pmt, axis=AX.X
                    )

                # ---- spatial conv (10 accumulated matmuls) + sigmoid ----
                psp = pss.tile([128, 32], f32, tag="sm")
                for ch in range(2):
                    mm = mp_map if ch == 0 else ap_map
                    for di in range(5):
                        nc.tensor.matmul(
                            psp,
                            wd_t[:, ch * 5 + di, :],
                            mm[:, di : di + 32],
                            start=(ch == 0 and di == 0),
                            stop=(ch == 1 and di == 4),
                        )
                sp_sig = work.tile([128, 32], f32, tag="spsig", bufs=3)
                nc.scalar.activation(
                    out=sp_sig, in_=psp, func=ACT.Sigmoid, bias=bias_t[:, 0:1]
                )

                # ---- sp -> [1, 4096] rows (hi/lo bf16) via transpose+flatten ----
                # f16 hi/lo split of sp (hi + lo recovers sp to ~1e-7)
                sph = work.tile([128, 32], f16, tag="sph", bufs=3)
                nc.vector.tensor_copy(sph, sp_sig)
                resid = work.tile([128, 32], f32, tag="resid", bufs=3)
                nc.vector.tensor_sub(resid, sp_sig, sph)
                spl = work.tile([128, 32], f16, tag="spl", bufs=3)
                nc.vector.tensor_copy(spl, resid)
                pspTh = pss.tile([32, 128], f16, tag="sm")
                nc.tensor.transpose(out=pspTh, in_=sph, identity=i128b_t)
                pspTl = pss.tile([32, 128], f16, tag="sm")
                nc.tensor.transpose(out=pspTl, in_=spl, identity=i128b_t)
                spc = work.tile([32, 2, 128], f16, tag="spc", bufs=3)
                nc.vector.tensor_copy(spc[:, 0, :], pspTh)
                nc.vector.tensor_copy(spc[:, 1, :], pspTl)
                # single contiguous flatten DMA:
                # flat[0, h2*256 + t*128 + hpw] = spc[h2, t, hpw]
                flat = flatp.tile([1, 2 * HW], f16, tag="flat")
                nc.scalar.dma_start(out=flat[:], in_=spc)
                fv = flat.rearrange("o (a t b) -> o a t b", t=2, b=128)

                # ---- ECA conv1d (banded matmul) + sigmoid + 1 ----
                ybh = work.tile([128, 2], f16, tag="ybh", bufs=3)
                nc.vector.tensor_copy(ybh, yb)
                pchw = pss.tile([128, 2], f32, tag="sm")
                for hp in range(2):
                    for kh in range(2):
                        nc.tensor.matmul(
                            pchw[:, hp : hp + 1],
                            w1t_t[:, kh, hp * 128 : (hp + 1) * 128],
                            ybh[:, kh : kh + 1],
                            start=(kh == 0),
                            stop=(kh == 1),
                        )
                chw1 = work.tile([128, 2], f32, tag="chw", bufs=3)
                nc.scalar.activation(out=chw1, in_=pchw, func=ACT.Sigmoid)
                nc.vector.tensor_scalar_add(chw1, chw1, 1.0)

                # ---- S = broadcast(sp) (PE); out = (S + chw1) * x (DVE fused) ----
                for q in range(4):
                    ps = psb.tile([128, 1024], f32, tag="big")
                    for seg in range(2):
                        h2a = q * 8 + seg * 4
                        o = ps[:, seg * 512 : (seg + 1) * 512]
                        nc.tensor.matmul(
                            o, ones1_t[0:1, :], fv[:, h2a : h2a + 4, 0, :],
                            start=True, stop=False,
                        )
                        nc.tensor.matmul(
                            o, ones1_t[0:1, :], fv[:, h2a : h2a + 4, 1, :],
                            start=False, stop=True,
                        )
                    for h in range(2):
                        xsl = x_t[:, h, q * 1024 : (q + 1) * 1024]
                        nc.vector.scalar_tensor_tensor(
                            out=xsl,
                            in0=ps,
                            scalar=chw1[:, h : h + 1],
                            in1=xsl,
                            op0=ALU.add,
                            op1=ALU.mult,
                        )

                # ---- store ----
                nc.scalar.dma_start(
                    out=out_d[b].rearrange("(g c) h w -> c g (h w)", g=2), in_=x_t
                )

    _split_excess_waits(nc, mybir)
    return nc


def _split_excess_waits(nc, mybir):
    """Walrus limits sync-wait commands per instruction (1 for compute
    engine instructions, ~2 for DMA).  Tile can emit more when an
    instruction depends on several engines.  Move the excess waits onto an
    inserted same-engine NoOp immediately before the instruction — engine
    program order makes this equivalent."""
    SKIP = (mybir.InstNoOp, mybir.InstAllEngineBarrier)
    for fn in nc.m.functions:
        for blk in fn.blocks:
            new = []
            for inst in blk.instructions:
                si = inst.sync_info
                if si is not None and si.on_wait and not isinstance(inst, SKIP):
                    waits = list(si.on_wait)
                    if len(waits) > 1:
                        moved, keep = waits[:-1], waits[-1:]
                        for k, w in enumerate(moved):
                            nop = mybir.InstNoOp(
                                name=f"{inst.name}-wsplit{k}",
                                engine=inst.engine,
                                sync_info=mybir.SyncInfo(on_wait=[w], on_update=[]),
                                bass_nofuse=True,
                            )
                            new.append(nop)
                        si.on_wait = keep
                new.append(inst)
            blk.instructions[:] = new


# ---------------------------------------------------------------------------
# Entry point
# ---------------------------------------------------------------------------

def kernel(x, conv1d_w, conv2d_w, conv2d_b):
    x = np.ascontiguousarray(np.asarray(x, np.float32))
    w1t, wd, bias = _build_host_consts(conv1d_w, conv2d_w, conv2d_b)

    from concourse.bass_utils import run_bass_kernel_spmd

    nc = _build_nc()
    shards = x.reshape(N_CORES, BPC, C, H, W)
    in_maps = [
        {"xs": np.ascontiguousarray(shards[i]), "w1t": w1t, "wd": wd, "bias": bias}
        for i in range(N_CORES)
    ]
    res = run_bass_kernel_spmd(nc, in_maps, core_ids=list(range(N_CORES)))
    out = np.concatenate([r["out"] for r in res.results], axis=0)
    return out.reshape(B, C, H, W)



# revision 3
# speedup vs baseline: 1.0656x; 1.0656x over previous
"""Trainium2 Bass kernel for CSHA attention (ECA channel + spatial attention).

out = x * (1 + ch_w[c] + sp[h, w]) for x [B, C, H, W] = [32, 256, 64, 64].

Per core (4 batches):
  ACT : cast x f32->f16 (+ accum_out channel sums for ECA), S' PSUM->SBUF
        chunk copies with bias = 1 + chw_g[c] fused in
  DVE : g-folds (max / in-place add), TRANSPOSE_TENSOR_REDUCE (32-group
        partition reduction), small f16 map-building ops, 1 of 4 final
        multiply chunks (stt from PSUM)
  PE  : 7x7 conv (10 banded matmuls), ECA conv1d, sp transpose, S'
        broadcast via ones-matmul (f16)
  Pool: 3 of 4 final multiply chunks (tensor_mul, in place over x)
  DMA : all 8 batch-half loads dispatched first; per-chunk stores
"""

import sys

import numpy as np

sys.path.insert(0, "/opt/trn_rl_repo")

B, C, H, W = 32, 256, 64, 64
HW = H * W            # 4096
N_CORES = 8
BPC = B // N_CORES    # 4 batches per core


def _build_host_consts(conv1d_w, conv2d_w, conv2d_b):
    w5 = np.asarray(conv1d_w, np.float32)[0, 0]           # [5]
    W1 = np.zeros((C, C), np.float32)
    for k in range(5):
        co = np.arange(C)
        ci = co + k - 2
        m = (ci >= 0) & (ci < C)
        W1[co[m], ci[m]] = w5[k] / HW
    w1t = W1.T.reshape(2, 128, C).transpose(1, 0, 2).astype(np.float16)

    wt = np.asarray(conv2d_w, np.float32)[0].copy()       # [2, 7, 7]
    wt[1] /= C
    wd = np.zeros((128, 10, 128), np.float32)
    wi = np.arange(64)
    for ch in range(2):
        for di, d in enumerate(range(-2, 3)):
            M = np.zeros((128, 128), np.float32)
            for hp in range(2):
                for hpp in range(2):
                    dy = 2 * d + hp - hpp + 3
                    if not (0 <= dy <= 6):
                        continue
                    for dx in range(7):
                        w_out = wi
                        w_in = w_out + dx - 3
                        msk = (w_in >= 0) & (w_in < 64)
                        M[hp * 64 + w_in[msk], hpp * 64 + w_out[msk]] = wt[ch, dy, dx]
            wd[:, ch * 5 + di, :] = M
    bias = np.full((128, 1), float(np.asarray(conv2d_b)[0]), np.float32)
    return w1t, wd.astype(np.float16), bias


def _build_nc():
    import concourse.bass as bass
    import concourse.tile as tile
    from concourse import mybir

    f32 = mybir.dt.float32
    f16 = mybir.dt.float16

    nc = bass.Bass()

    xs_d = nc.dram_tensor("xs", [BPC, C, H, W], f32, kind="ExternalInput")
    w1t_d = nc.dram_tensor("w1t", [128, 2, C], f16, kind="ExternalInput")
    wd_d = nc.dram_tensor("wd", [128, 10, 128], f16, kind="ExternalInput")
    bias_d = nc.dram_tensor("bias", [128, 1], f32, kind="ExternalInput")
    out_d = nc.dram_tensor("out", [BPC, C, H, W], f32, kind="ExternalOutput")

    i128b_d = nc.inline_tensor(np.eye(128, dtype=np.float16), "i128b")
    ones1_d = nc.inline_tensor(np.ones((1, 128), np.float16), "ones1")

    AX = mybir.AxisListType
    ALU = mybir.AluOpType
    ACT = mybir.ActivationFunctionType

    with tile.TileContext(nc) as tc:
        with (
            tc.tile_pool(name="consts", bufs=1) as consts,
            tc.tile_pool(name="xp", bufs=4) as xp,
            tc.tile_pool(name="scrp", bufs=2) as scrp,
            tc.tile_pool(name="m1p", bufs=1) as m1p,
            tc.tile_pool(name="work", bufs=2) as work,
            tc.tile_pool(name="mapp", bufs=1) as mapp,
            tc.tile_pool(name="rrp", bufs=2) as rrp,
            tc.tile_pool(name="ssb", bufs=1) as ssb,
            tc.tile_pool(name="psb", bufs=3, space=bass.MemorySpace.PSUM) as psb,
            tc.tile_pool(name="pss", bufs=2, space=bass.MemorySpace.PSUM) as pss,
        ):
            # ---- constants first (tiny; must not queue behind the bulk
            # x loads — PE conv/bcast and ACT sigmoid all gate on them) ----
            w1t_t = consts.tile([128, 2, C], f16)
            nc.sync.dma_start(out=w1t_t, in_=w1t_d[:])
            wd_t = consts.tile([128, 10, 128], f16)
            nc.sync.dma_start(out=wd_t, in_=wd_d[:])
            bias_t = consts.tile([128, 1], f32)
            nc.sync.dma_start(out=bias_t, in_=bias_d[:])
            i128b_t = consts.tile([128, 128], f16)
            nc.sync.dma_start(out=i128b_t, in_=i128b_d[:])
            ones1_t = consts.tile([1, 128], f16)
            nc.sync.dma_start(out=ones1_t, in_=ones1_d[:])

            # ---- all batch loads dispatched up front (per g half) ----
            x_tiles = []
            for b in range(BPC):
                x_t = xp.tile([128, 2, HW], f32, tag="x")
                x_tiles.append(x_t)
                for u in range(2):
                    for g in range(2):
                        nc.sync.dma_start(
                            out=x_t[:, g, u * 2048 : (u + 1) * 2048],
                            in_=xs_d[b, 128 * g : 128 * (g + 1)].rearrange(
                                "c h w -> c (h w)"
                            )[:, u * 2048 : (u + 1) * 2048],
                        )

            # Dummy matmuls absorb const-load DMA waits on PE early.
            pd = pss.tile([1, 4], f32, tag="sm")
            for k, (lhs, rhs) in enumerate((
                (w1t_t[:, 0, 0:1], w1t_t[:, 0, 1:2]),
                (wd_t[:, 0, 0:1], wd_t[:, 0, 1:2]),
                (i128b_t[:, 0:1], i128b_t[:, 1:2]),
            )):
                nc.tensor.matmul(pd[:, k : k + 1], lhs, rhs, start=True, stop=True)
            junk0 = work.tile([128, 1], f32, tag="junk")
            nc.scalar.activation(out=junk0, in_=bias_t, func=ACT.Copy)

            # Interleaved conv maps: pads (cols 0:2, 34:36) zeroed once;
            # data cols rewritten every batch.
            mp_map = mapp.tile([128, 36], f16, tag="mp")
            ap_map = mapp.tile([128, 36], f16, tag="ap")
            for mp in (mp_map, ap_map):
                nc.vector.memset(
                    mp.rearrange("p (u c) -> p u c", u=18)[:, 0:18:17, :], 0.0
                )

            for b in range(BPC):
                x_t = x_tiles[b]

                # ---- ACT cast f32->f16 + ECA channel sums over hw ----
                # Split into 2048-wide pieces: long ACT ops otherwise block
                # the (high-priority) tail ops of the previous batch.
                scr = scrp.tile([128, 2, HW], f16, tag="scr")
                yb8 = work.tile([128, 2, 2], f32, tag="yb8")
                for u in range(2):
                    for g in range(2):
                        sl = slice(u * 2048, (u + 1) * 2048)
                        nc.scalar.activation(
                            out=scr[:, g, sl],
                            in_=x_t[:, g, sl],
                            func=ACT.Copy,
                            accum_out=yb8[:, g, u : u + 1],
                        )
                yb = work.tile([128, 2], f32, tag="yb")
                nc.vector.tensor_add(yb, yb8[:, :, 0], yb8[:, :, 1])

                # ---- DVE g-folds (split in halves, same reason) ----
                m1 = m1p.tile([128, HW], f16, tag="m1")
                for u in range(2):
                    sl = slice(u * 2048, (u + 1) * 2048)
                    nc.vector.tensor_max(m1[:, sl], scr[:, 0, sl], scr[:, 1, sl])

                # ---- transpose-reduce: 32-group partition reduction ----
                # R[32a+i, v] = red_j in[32a+j, 32v+i]   (pixel hw = 32v+i)
                # Split by v-halves for scheduling granularity.
                Rm = work.tile([128, 128], f16, tag="Rm")
                Ra = work.tile([128, 128], f32, tag="Ra")
                for u in range(2):
                    nc.vector.tensor_reduce(
                        out=Rm[:, u * 64 : (u + 1) * 64],
                        in_=m1[:, u * 2048 : (u + 1) * 2048].rearrange(
                            "p (v j) -> p v j", j=32
                        ),
                        axis=AX.X, op=ALU.max, apply_transpose=True,
                    )
                for u in range(2):
                    sl = slice(u * 2048, (u + 1) * 2048)
                    nc.vector.tensor_add(scr[:, 0, sl], scr[:, 0, sl], scr[:, 1, sl])
                for u in range(2):
                    nc.vector.tensor_reduce(
                        out=Ra[:, u * 64 : (u + 1) * 64],
                        in_=scr[:, 0, u * 2048 : (u + 1) * 2048].rearrange(
                            "p (v j) -> p v j", j=32
                        ),
                        axis=AX.X, op=ALU.add, apply_transpose=True,
                    )

                # ---- finish reduction over the 4 a-groups; build maps ----
                # map[32q+i, 2+h2] = red_a R[32a+i, 4h2+q]
                # The whole per-batch tail (small map ops -> conv -> S' ->
                # multiply -> store) runs at high priority so the scheduler
                # drains batch b's latency chain before batch b+1's bulk
                # fold/reduce work.
                tail_ctx = tc.high_priority()
                tail_ctx.__enter__()
                for path, (R, top) in enumerate(((Rm, ALU.max), (Ra, ALU.add))):
                    G = work.tile([32, 4, 128], f16, tag=f"G{path}")
                    for a in range(4):
                        nc.scalar.activation(
                            out=G[:, a, :], in_=R[32 * a : 32 * a + 32, :],
                            func=ACT.Copy,
                        )
                    G2 = work.tile([32, 2, 128], f16, tag=f"G2{path}")
                    nc.vector.tensor_tensor(
                        out=G2.rearrange("p t v -> p (t v)"),
                        in0=G[:, 0:2, :].rearrange("p t v -> p (t v)"),
                        in1=G[:, 2:4, :].rearrange("p t v -> p (t v)"),
                        op=top,
                    )
                    # Final fold writes map stripes directly (output at
                    # partition base 32q): map[32q+i, 2+h2] = red over t of
                    # G2[i, t, 4h2+q].
                    mp = mp_map if path == 0 else ap_map
                    for q in range(4):
                        nc.vector.tensor_tensor(
                            out=mp[32 * q : 32 * q + 32, 2:34],
                            in0=G2[:, 0, q : 125 + q : 4],
                            in1=G2[:, 1, q : 125 + q : 4],
                            op=top,
                        )

                # ---- spatial conv (10 accumulated matmuls) + sigmoid ----
                psp = pss.tile([128, 32], f32, tag="sm")
                for ch in range(2):
                    mm = mp_map if ch == 0 else ap_map
                    for di in range(5):
                        nc.tensor.matmul(
                            psp,
                            wd_t[:, ch * 5 + di, :],
                            mm[:, di : di + 32],
                            start=(ch == 0 and di == 0),
                            stop=(ch == 1 and di == 4),
                        )
                sp16 = work.tile([128, 32], f16, tag="sp16")
                nc.scalar.activation(
                    out=sp16, in_=psp, func=ACT.Sigmoid, bias=bias_t[:, 0:1]
                )

                # ---- sp row: transpose + flatten DMA -> rrow [1, HW] ----
                pspT = pss.tile([32, 128], f16, tag="sm")
                nc.tensor.transpose(out=pspT, in_=sp16, identity=i128b_t)
                fr = work.tile([32, 128], f16, tag="fr")
                nc.scalar.activation(out=fr, in_=pspT, func=ACT.Identity, bias=1.0)
                rrow = rrp.tile([1, HW], f16, tag="rrow")
                nc.sync.dma_start(out=rrow, in_=fr)

                # ---- ECA conv1d + sigmoid -> chw1 = 1 + sigmoid(...) ----
                ybh = work.tile([128, 2], f16, tag="ybh")
                nc.scalar.activation(out=ybh, in_=yb, func=ACT.Copy)
                pchw = pss.tile([128, 2], f32, tag="sm")
                for hp in range(2):
                    for kh in range(2):
                        nc.tensor.matmul(
                            pchw[:, hp : hp + 1],
                            w1t_t[:, kh, hp * 128 : (hp + 1) * 128],
                            ybh[:, kh : kh + 1],
                            start=(kh == 0),
                            stop=(kh == 1),
                        )
                chw1 = work.tile([128, 2], f32, tag="chw")
                nc.scalar.activation(out=chw1, in_=pchw, func=ACT.Sigmoid)

                # ---- S' broadcast (PE, per 1024) + DVE stt multiply ----
                # stt: out = (S_psum + chw1_g) * x, in place over x.  All
                # multiplies on DVE (the dominant stream); no SBUF S' copies.
                tail_ctx.__exit__(None, None, None)
                for g in range(2):
                    for cc in range(2):
                        if g == 1 and cc == 1:
                            # Pool chunk: PSUM -> SBUF via ACT (bias adds
                            # 1 + chw), gpsimd multiply, store
                            xsl = x_t[:, g, cc * 2048 : (cc + 1) * 2048]
                            Sb = ssb.tile([128, 2048], f32, tag="Sb")
                            for half in range(2):
                                ps = psb.tile([128, 1024], f32, tag="bc")
                                for k in range(2):
                                    col = cc * 2048 + half * 1024 + k * 512
                                    nc.tensor.matmul(
                                        ps[:, k * 512 : (k + 1) * 512],
                                        ones1_t,
                                        rrow[:, col : col + 512],
                                        start=True, stop=True,
                                    )
                                nc.scalar.activation(
                                    out=Sb[:, half * 1024 : (half + 1) * 1024],
                                    in_=ps,
                                    func=ACT.Identity,
                                    bias=chw1[:, g : g + 1],
                                )
                            nc.gpsimd.tensor_mul(xsl, xsl, Sb)
                            nc.sync.dma_start(
                                out=out_d[b, 128 * g : 128 * (g + 1)]
                                .rearrange("c h w -> c (h w)")
                                [:, cc * 2048 : (cc + 1) * 2048],
                                in_=xsl,
                            )
                            continue
                        for half in range(2):
                            ps = psb.tile([128, 1024], f32, tag="bc")
                            for k in range(2):
                                col = cc * 2048 + half * 1024 + k * 512
                                nc.tensor.matmul(
                                    ps[:, k * 512 : (k + 1) * 512],
                                    ones1_t,
                                    rrow[:, col : col + 512],
                                    start=True, stop=True,
                                )
                            xh = x_t[:, g, cc * 2048 + half * 1024 :
                                     cc * 2048 + (half + 1) * 1024]
                            nc.vector.scalar_tensor_tensor(
                                out=xh,
                                in0=ps,
                                scalar=chw1[:, g : g + 1],
                                in1=xh,
                                op0=ALU.add,
                                op1=ALU.mult,
                            )
                        nc.sync.dma_start(
                            out=out_d[b, 128 * g : 128 * (g + 1)]
                            .rearrange("c h w -> c (h w)")
                            [:, cc * 2048 : (cc + 1) * 2048],
                            in_=x_t[:, g, cc * 2048 : (cc + 1) * 2048],
                        )

    _split_excess_waits(nc, mybir)
    return nc


def _split_excess_waits(nc, mybir):
    """Walrus limits sync-wait commands per instruction.  Move excess waits
    onto an inserted same-engine NoOp immediately before the instruction."""
    SKIP = (mybir.InstNoOp, mybir.InstAllEngineBarrier)
    for fn in nc.m.functions:
        for blk in fn.blocks:
            new = []
            for inst in blk.instructions:
                si = inst.sync_info
                if si is not None and si.on_wait and not isinstance(inst, SKIP):
                    waits = list(si.on_wait)
                    if len(waits) > 1:
                        moved, keep = waits[:-1], waits[-1:]
                        for k, w in enumerate(moved):
                            nop = mybir.InstNoOp(
                                name=f"{inst.name}-wsplit{k}",
                                engine=inst.engine,
                                sync_info=mybir.SyncInfo(on_wait=[w], on_update=[]),
                                bass_nofuse=True,
                            )
                            new.append(nop)
                        si.on_wait = keep
                new.append(inst)
            blk.instructions[:] = new


def kernel(x, conv1d_w, conv2d_w, conv2d_b):
    x = np.ascontiguousarray(np.asarray(x, np.float32))
    w1t, wd, bias = _build_host_consts(conv1d_w, conv2d_w, conv2d_b)

    from concourse.bass_utils import run_bass_kernel_spmd

    nc = _build_nc()
    shards = x.reshape(N_CORES, BPC, C, H, W)
    in_maps = [
        {"xs": np.ascontiguousarray(shards[i]), "w1t": w1t, "wd": wd, "bias": bias}
        for i in range(N_CORES)
    ]
    res = run_bass_kernel_spmd(nc, in_maps, core_ids=list(range(N_CORES)))
    out = np.concatenate([r["out"] for r in res.results], axis=0)
    return out.reshape(B, C, H, W)


# revision 4
# speedup vs baseline: 1.1006x; 1.0329x over previous
"""Trainium2 Bass kernel for CSHA attention (ECA channel + spatial attention).

out = x * (1 + ch_w[c] + sp[h, w]) for x [B, C, H, W] = [32, 256, 64, 64].

Per core (4 batches):
  ACT : cast x f32->f16 (+ accum_out channel sums for ECA), S' PSUM->SBUF
        chunk copies with bias = 1 + chw_g[c] fused in
  DVE : g-folds (max / in-place add), TRANSPOSE_TENSOR_REDUCE (32-group
        partition reduction), small f16 map-building ops, 1 of 4 final
        multiply chunks (stt from PSUM)
  PE  : 7x7 conv (10 banded matmuls), ECA conv1d, sp transpose, S'
        broadcast via ones-matmul (f16)
  Pool: 3 of 4 final multiply chunks (tensor_mul, in place over x)
  DMA : all 8 batch-half loads dispatched first; per-chunk stores
"""

import sys

import numpy as np

sys.path.insert(0, "/opt/trn_rl_repo")

B, C, H, W = 32, 256, 64, 64
HW = H * W            # 4096
N_CORES = 8
BPC = B // N_CORES    # 4 batches per core


def _build_host_consts(conv1d_w, conv2d_w, conv2d_b):
    w5 = np.asarray(conv1d_w, np.float32)[0, 0]           # [5]
    W1 = np.zeros((C, C), np.float32)
    for k in range(5):
        co = np.arange(C)
        ci = co + k - 2
        m = (ci >= 0) & (ci < C)
        W1[co[m], ci[m]] = w5[k] / HW
    w1t = W1.T.reshape(2, 128, C).transpose(1, 0, 2).astype(np.float16)

    wt = np.asarray(conv2d_w, np.float32)[0].copy()       # [2, 7, 7]
    wt[1] /= C
    wd = np.zeros((128, 10, 128), np.float32)
    wi = np.arange(64)
    for ch in range(2):
        for di, d in enumerate(range(-2, 3)):
            M = np.zeros((128, 128), np.float32)
            for hp in range(2):
                for hpp in range(2):
                    dy = 2 * d + hp - hpp + 3
                    if not (0 <= dy <= 6):
                        continue
                    for dx in range(7):
                        w_out = wi
                        w_in = w_out + dx - 3
                        msk = (w_in >= 0) & (w_in < 64)
                        M[hp * 64 + w_in[msk], hpp * 64 + w_out[msk]] = wt[ch, dy, dx]
            wd[:, ch * 5 + di, :] = M
    bias = np.full((128, 1), float(np.asarray(conv2d_b)[0]), np.float32)
    return w1t, wd.astype(np.float16), bias


def _build_nc():
    import concourse.bass as bass
    import concourse.tile as tile
    from concourse import mybir

    f32 = mybir.dt.float32
    f16 = mybir.dt.float16

    nc = bass.Bass()

    xs_d = nc.dram_tensor("xs", [BPC, C, H, W], f32, kind="ExternalInput")
    w1t_d = nc.dram_tensor("w1t", [128, 2, C], f16, kind="ExternalInput")
    wd_d = nc.dram_tensor("wd", [128, 10, 128], f16, kind="ExternalInput")
    bias_d = nc.dram_tensor("bias", [128, 1], f32, kind="ExternalInput")
    out_d = nc.dram_tensor("out", [BPC, C, H, W], f32, kind="ExternalOutput")

    i128b_d = nc.inline_tensor(np.eye(128, dtype=np.float16), "i128b")
    ones1_d = nc.inline_tensor(np.ones((1, 128), np.float16), "ones1")

    AX = mybir.AxisListType
    ALU = mybir.AluOpType
    ACT = mybir.ActivationFunctionType

    with tile.TileContext(nc) as tc:
        with (
            tc.tile_pool(name="consts", bufs=1) as consts,
            tc.tile_pool(name="xp", bufs=4) as xp,
            tc.tile_pool(name="scrp", bufs=2) as scrp,
            tc.tile_pool(name="m1p", bufs=1) as m1p,
            tc.tile_pool(name="work", bufs=2) as work,
            tc.tile_pool(name="mapp", bufs=1) as mapp,
            tc.tile_pool(name="rrp", bufs=2) as rrp,
            tc.tile_pool(name="ssb", bufs=1) as ssb,
            tc.tile_pool(name="psb", bufs=3, space=bass.MemorySpace.PSUM) as psb,
            tc.tile_pool(name="pss", bufs=2, space=bass.MemorySpace.PSUM) as pss,
        ):
            # ---- constants first (tiny; must not queue behind the bulk
            # x loads — PE conv/bcast and ACT sigmoid all gate on them) ----
            w1t_t = consts.tile([128, 2, C], f16)
            nc.sync.dma_start(out=w1t_t, in_=w1t_d[:])
            wd_t = consts.tile([128, 10, 128], f16)
            nc.sync.dma_start(out=wd_t, in_=wd_d[:])
            bias_t = consts.tile([128, 1], f32)
            nc.sync.dma_start(out=bias_t, in_=bias_d[:])
            i128b_t = consts.tile([128, 128], f16)
            nc.sync.dma_start(out=i128b_t, in_=i128b_d[:])
            ones1_t = consts.tile([1, 128], f16)
            nc.sync.dma_start(out=ones1_t, in_=ones1_d[:])

            # ---- all batch loads dispatched up front (per g half) ----
            x_tiles = []
            for b in range(BPC):
                x_t = xp.tile([128, 2, HW], f32, tag="x")
                x_tiles.append(x_t)
                for u in range(2):
                    for g in range(2):
                        nc.sync.dma_start(
                            out=x_t[:, g, u * 2048 : (u + 1) * 2048],
                            in_=xs_d[b, 128 * g : 128 * (g + 1)].rearrange(
                                "c h w -> c (h w)"
                            )[:, u * 2048 : (u + 1) * 2048],
                        )

            # Dummy matmuls absorb const-load DMA waits on PE early.
            pd = pss.tile([1, 4], f32, tag="sm")
            for k, (lhs, rhs) in enumerate((
                (w1t_t[:, 0, 0:1], w1t_t[:, 0, 1:2]),
                (wd_t[:, 0, 0:1], wd_t[:, 0, 1:2]),
                (i128b_t[:, 0:1], i128b_t[:, 1:2]),
            )):
                nc.tensor.matmul(pd[:, k : k + 1], lhs, rhs, start=True, stop=True)
            junk0 = work.tile([128, 1], f32, tag="junk")
            nc.scalar.activation(out=junk0, in_=bias_t, func=ACT.Copy)

            # Interleaved conv maps: pads (cols 0:2, 34:36) zeroed once;
            # data cols rewritten every batch.
            mp_map = mapp.tile([128, 36], f16, tag="mp")
            ap_map = mapp.tile([128, 36], f16, tag="ap")
            for mp in (mp_map, ap_map):
                nc.vector.memset(
                    mp.rearrange("p (u c) -> p u c", u=18)[:, 0:18:17, :], 0.0
                )

            for b in range(BPC):
                x_t = x_tiles[b]

                # ---- ACT cast f32->f16 + ECA channel sums over hw ----
                # Split into 2048-wide pieces: long ACT ops otherwise block
                # the (high-priority) tail ops of the previous batch.
                scr = scrp.tile([128, 2, HW], f16, tag="scr")
                yb8 = work.tile([128, 2, 2], f32, tag="yb8")
                for u in range(2):
                    for g in range(2):
                        sl = slice(u * 2048, (u + 1) * 2048)
                        nc.scalar.activation(
                            out=scr[:, g, sl],
                            in_=x_t[:, g, sl],
                            func=ACT.Copy,
                            accum_out=yb8[:, g, u : u + 1],
                        )
                yb = work.tile([128, 2], f32, tag="yb")
                nc.vector.tensor_add(yb, yb8[:, :, 0], yb8[:, :, 1])

                # ---- DVE g-folds (split in halves, same reason) ----
                m1 = m1p.tile([128, HW], f16, tag="m1")
                for u in range(2):
                    sl = slice(u * 2048, (u + 1) * 2048)
                    nc.vector.tensor_max(m1[:, sl], scr[:, 0, sl], scr[:, 1, sl])

                # ---- transpose-reduce: 32-group partition reduction ----
                # R[32a+i, v] = red_j in[32a+j, 32v+i]   (pixel hw = 32v+i)
                # Split by v-halves for scheduling granularity.
                Rm = work.tile([128, 128], f16, tag="Rm")
                Ra = work.tile([128, 128], f32, tag="Ra")
                for u in range(2):
                    nc.vector.tensor_reduce(
                        out=Rm[:, u * 64 : (u + 1) * 64],
                        in_=m1[:, u * 2048 : (u + 1) * 2048].rearrange(
                            "p (v j) -> p v j", j=32
                        ),
                        axis=AX.X, op=ALU.max, apply_transpose=True,
                    )
                for u in range(2):
                    sl = slice(u * 2048, (u + 1) * 2048)
                    nc.vector.tensor_add(scr[:, 0, sl], scr[:, 0, sl], scr[:, 1, sl])
                for u in range(2):
                    nc.vector.tensor_reduce(
                        out=Ra[:, u * 64 : (u + 1) * 64],
                        in_=scr[:, 0, u * 2048 : (u + 1) * 2048].rearrange(
                            "p (v j) -> p v j", j=32
                        ),
                        axis=AX.X, op=ALU.add, apply_transpose=True,
                    )

                # ---- finish reduction over the 4 a-groups; build maps ----
                # map[32q+i, 2+h2] = red_a R[32a+i, 4h2+q]
                # The whole per-batch tail (small map ops -> conv -> S' ->
                # multiply -> store) runs at high priority so the scheduler
                # drains batch b's latency chain before batch b+1's bulk
                # fold/reduce work.
                tail_ctx = tc.high_priority()
                tail_ctx.__enter__()
                for path, (R, top) in enumerate(((Rm, ALU.max), (Ra, ALU.add))):
                    G = work.tile([32, 4, 128], f16, tag=f"G{path}")
                    for a in range(4):
                        nc.scalar.activation(
                            out=G[:, a, :], in_=R[32 * a : 32 * a + 32, :],
                            func=ACT.Copy,
                        )
                    G2 = work.tile([32, 2, 128], f16, tag=f"G2{path}")
                    nc.vector.tensor_tensor(
                        out=G2.rearrange("p t v -> p (t v)"),
                        in0=G[:, 0:2, :].rearrange("p t v -> p (t v)"),
                        in1=G[:, 2:4, :].rearrange("p t v -> p (t v)"),
                        op=top,
                    )
                    # Final fold writes map stripes directly (output at
                    # partition base 32q): map[32q+i, 2+h2] = red over t of
                    # G2[i, t, 4h2+q].
                    mp = mp_map if path == 0 else ap_map
                    for q in range(4):
                        nc.vector.tensor_tensor(
                            out=mp[32 * q : 32 * q + 32, 2:34],
                            in0=G2[:, 0, q : 125 + q : 4],
                            in1=G2[:, 1, q : 125 + q : 4],
                            op=top,
                        )

                # ---- spatial conv (10 accumulated matmuls) + sigmoid ----
                psp = pss.tile([128, 32], f32, tag="sm")
                for ch in range(2):
                    mm = mp_map if ch == 0 else ap_map
                    for di in range(5):
                        nc.tensor.matmul(
                            psp,
                            wd_t[:, ch * 5 + di, :],
                            mm[:, di : di + 32],
                            start=(ch == 0 and di == 0),
                            stop=(ch == 1 and di == 4),
                        )
                sp16 = work.tile([128, 32], f16, tag="sp16")
                nc.scalar.activation(
                    out=sp16, in_=psp, func=ACT.Sigmoid, bias=bias_t[:, 0:1]
                )

                # ---- sp row: transpose + flatten DMA -> rrow [1, HW] ----
                pspT = pss.tile([32, 128], f16, tag="sm")
                nc.tensor.transpose(out=pspT, in_=sp16, identity=i128b_t)
                fr = work.tile([32, 128], f16, tag="fr")
                nc.scalar.activation(out=fr, in_=pspT, func=ACT.Identity, bias=1.0)
                rrow = rrp.tile([1, HW], f16, tag="rrow")
                nc.sync.dma_start(out=rrow, in_=fr)

                # ---- ECA conv1d + sigmoid -> chw1 = 1 + sigmoid(...) ----
                ybh = work.tile([128, 2], f16, tag="ybh")
                nc.scalar.activation(out=ybh, in_=yb, func=ACT.Copy)
                pchw = pss.tile([128, 2], f32, tag="sm")
                for hp in range(2):
                    for kh in range(2):
                        nc.tensor.matmul(
                            pchw[:, hp : hp + 1],
                            w1t_t[:, kh, hp * 128 : (hp + 1) * 128],
                            ybh[:, kh : kh + 1],
                            start=(kh == 0),
                            stop=(kh == 1),
                        )
                chw1 = work.tile([128, 2], f32, tag="chw")
                nc.scalar.activation(out=chw1, in_=pchw, func=ACT.Sigmoid)

                # ---- S' broadcast (PE, per 1024) + DVE stt multiply ----
                # stt: out = (S_psum + chw1_g) * x, in place over x.  All
                # multiplies on DVE (the dominant stream); no SBUF S' copies.
                tail_ctx.__exit__(None, None, None)
                for g in range(2):
                    for cc in range(2):
                        if g == 1 and cc == 1 and b < BPC - 1:
                            # Pool chunk: PSUM -> SBUF via ACT (bias adds
                            # 1 + chw), gpsimd multiply, store
                            xsl = x_t[:, g, cc * 2048 : (cc + 1) * 2048]
                            Sb = ssb.tile([128, 2048], f32, tag="Sb")
                            for half in range(2):
                                ps = psb.tile([128, 1024], f32, tag="bc")
                                for k in range(2):
                                    col = cc * 2048 + half * 1024 + k * 512
                                    nc.tensor.matmul(
                                        ps[:, k * 512 : (k + 1) * 512],
                                        ones1_t,
                                        rrow[:, col : col + 512],
                                        start=True, stop=True,
                                    )
                                nc.scalar.activation(
                                    out=Sb[:, half * 1024 : (half + 1) * 1024],
                                    in_=ps,
                                    func=ACT.Identity,
                                    bias=chw1[:, g : g + 1],
                                )
                            nc.gpsimd.tensor_mul(xsl, xsl, Sb)
                            nc.sync.dma_start(
                                out=out_d[b, 128 * g : 128 * (g + 1)]
                                .rearrange("c h w -> c (h w)")
                                [:, cc * 2048 : (cc + 1) * 2048],
                                in_=xsl,
                            )
                            continue
                        for half in range(2):
                            ps = psb.tile([128, 1024], f32, tag="bc")
                            for k in range(2):
                                col = cc * 2048 + half * 1024 + k * 512
                                nc.tensor.matmul(
                                    ps[:, k * 512 : (k + 1) * 512],
                                    ones1_t,
                                    rrow[:, col : col + 512],
                                    start=True, stop=True,
                                )
                            xh = x_t[:, g, cc * 2048 + half * 1024 :
                                     cc * 2048 + (half + 1) * 1024]
                            nc.vector.scalar_tensor_tensor(
                                out=xh,
                                in0=ps,
                                scalar=chw1[:, g : g + 1],
                                in1=xh,
                                op0=ALU.add,
                                op1=ALU.mult,
                            )
                        nc.sync.dma_start(
                            out=out_d[b, 128 * g : 128 * (g + 1)]
                            .rearrange("c h w -> c (h w)")
                            [:, cc * 2048 : (cc + 1) * 2048],
                            in_=x_t[:, g, cc * 2048 : (cc + 1) * 2048],
                        )

    _split_excess_waits(nc, mybir)
    return nc


def _split_excess_waits(nc, mybir):
    """Walrus limits sync-wait commands per instruction.  Move excess waits
    onto an inserted same-engine NoOp immediately before the instruction."""
    SKIP = (mybir.InstNoOp, mybir.InstAllEngineBarrier)
    for fn in nc.m.functions:
        for blk in fn.blocks:
            new = []
            for inst in blk.instructions:
                si = inst.sync_info
                if si is not None and si.on_wait and not isinstance(inst, SKIP):
                    waits = list(si.on_wait)
                    if len(waits) > 1:
                        moved, keep = waits[:-1], waits[-1:]
                        for k, w in enumerate(moved):
                            nop = mybir.InstNoOp(
                                name=f"{inst.name}-wsplit{k}",
                                engine=inst.engine,
                                sync_info=mybir.SyncInfo(on_wait=[w], on_update=[]),
                                bass_nofuse=True,
                            )
                            new.append(nop)
                        si.on_wait = keep
                new.append(inst)
            blk.instructions[:] = new


def kernel(x, conv1d_w, conv2d_w, conv2d_b):
    x = np.ascontiguousarray(np.asarray(x, np.float32))
    w1t, wd, bias = _build_host_consts(conv1d_w, conv2d_w, conv2d_b)

    from concourse.bass_utils import run_bass_kernel_spmd

    nc = _build_nc()
    shards = x.reshape(N_CORES, BPC, C, H, W)
    in_maps = [
        {"xs": np.ascontiguousarray(shards[i]), "w1t": w1t, "wd": wd, "bias": bias}
        for i in range(N_CORES)
    ]
    res = run_bass_kernel_spmd(nc, in_maps, core_ids=list(range(N_CORES)))
    out = np.concatenate([r["out"] for r in res.results], axis=0)
    return out.reshape(B, C, H, W)
